# revision 18
# baseline (speedup 1.0000x reference)
"""Trainium2 Bass kernel for nn_MlroleNode_64716567216639 (GAT message passing).

Math note: the reference computes a dense NxN GATv2 attention but only row 0
of the output (gat_out[0]) feeds the final MLP, so this kernel computes just
that row: e[j,h] = leaky(g_l[j] + g_r[0]) . w_attn over the 1024 source nodes,
softmax, weighted sum of g_r, then the 3-layer type-define MLP over the 1023
ambiguous nodes.

Structure: node 0's embedding h1 (serial role-routing prologue) is written
into column 1023 of the on-chip node matrix, so the per-source-node work is a
single [128,1024] pipeline; only three 1-column matmuls depend on the
prologue. All matmuls and large elementwise ops run in bf16 (fp32 PSUM
accumulation); biases are folded into augmented weight rows and the per-role
merge bias C = WmR@(Wt/3 @ sum(agents) + bt) + bm is collapsed into a single
matmul host-side. MLP leaky-relus use leaky(v) = 0.8*relu(v) + 0.2*v with the
0.8/0.2 folded into split weight copies so ACT (relu) and DVE (copy) run in
parallel. The sigmoid is computed as tanh on-device with the affine
0.5+0.5*x applied on host. The GAT row-0 computation is replicated on all 8
cores; the final MLP is sharded 128 nodes per core.
"""
import numpy as np

H = 64
N_AMB = 1023
N = 1024            # 1023 ambiguous + node 0 (in column 1023)
HEADS = 4
HID = 64
RT = 4
APT = 3
SLOPE = 0.2
NCORES = 8
SHARD = 128

# "pro1" [65, 129]: chain weights; "pro2" [65, 268]: folded C-path weights
PRO_WSELF = 0          # WselfT_aug [65,64]
PRO_WML = 64           # WmLT [64,64]
PRO_HID = 128          # hidc_aug [65,1]
PRO1_COLS = 129
PRO_WC = 0             # 4x WC_t_aug [65,64]: C_t = (WmR@Wt_t/3) @ tsum + bC_t
PRO_TA = 256           # ta [64,12]
PRO2_COLS = 268

# column offsets in the packed "gatw" tensor [64, 1536]
GW_AMB = 0             # ambT [64,1023] + h1 slot col 1023
GW_WL = 1024           # WlT [64,256] (2 blocks of 128)
GW_WR = 1280           # WrT [64,256]
GW_COLS = 1536

# column offsets in the packed "mlpw" tensor [128, 392]
MW_G = 0               # G = 0.25*[Wd0b.T; Wd0b.T] [128,64]
MW_WD1A = 64           # [0.8*Wd1.T; bd1] [65,128]
MW_WD1B = 192          # [0.2*Wd1.T; 0]  [65,128]
MW_WD0A = 320          # Wd0a_aug [65,64]
MW_WD2A = 384          # 0.8*Wd2.T [128,4]
MW_WD2B = 388          # 0.2*Wd2.T [128,4]
MW_COLS = 392

_compiled = None


def _build():
    import concourse.tile as tile
    from concourse import bacc, mybir

    f32 = mybir.dt.float32
    bf16 = mybir.dt.bfloat16
    AF = mybir.ActivationFunctionType
    ALU = mybir.AluOpType
    AX = mybir.AxisListType

    nc = bacc.Bacc("TRN2", target_bir_lowering=False, debug=False,
                   enable_asserts=False, num_devices=NCORES)

    pro1_d = nc.dram_tensor("pro1", [65, PRO1_COLS], bf16, kind="ExternalInput").ap()
    pro2_d = nc.dram_tensor("pro2", [65, PRO2_COLS], bf16, kind="ExternalInput").ap()
    gatw_d = nc.dram_tensor("gatw", [64, GW_COLS], bf16, kind="ExternalInput").ap()
    wexp_d = nc.dram_tensor("wexp", [128, 128], bf16, kind="ExternalInput").ap()
    mlpw_d = nc.dram_tensor("mlpw", [128, MW_COLS], bf16, kind="ExternalInput").ap()
    mlpin_d = nc.dram_tensor("mlpin", [65, SHARD], bf16, kind="ExternalInput").ap()
    bd2h_d = nc.dram_tensor("bd2h", [RT, 1], f32, kind="ExternalInput").ap()
    outT_d = nc.dram_tensor("outT", [RT, SHARD], f32, kind="ExternalOutput").ap()

    # column chunks: [0:512), [512:1023) independent of h1; col 1023 is h1
    CH = ((0, 512), (512, N_AMB))
    PAD_BIG = [128, 1024]   # 2 PSUM banks

    with tile.TileContext(nc) as tc:
        with tc.tile_pool(name="wp", bufs=1) as wp, \
             tc.tile_pool(name="sb", bufs=1) as sb, \
             tc.tile_pool(name="ps", bufs=1, space="PSUM") as ps:

            # ---- input DMAs, critical-first, split across two queues ----
            pro1 = wp.tile([65, PRO1_COLS], bf16, tag="pro1")
            nc.sync.dma_start(pro1[:], pro1_d[:])
            pro2 = wp.tile([65, PRO2_COLS], bf16, tag="pro2")
            nc.sync.dma_start(pro2[:], pro2_d[:])
            gatw = wp.tile([64, GW_COLS], bf16, tag="gatw")
            nc.sync.dma_start(gatw[:], gatw_d[:])
            wexp = wp.tile([128, 128], bf16, tag="wexp")
            nc.sync.dma_start(wexp[:], wexp_d[:])
            mlpin = wp.tile([65, SHARD], bf16, tag="mlpin")
            nc.gpsimd.dma_start(mlpin[:], mlpin_d[:])
            mlpw = wp.tile([128, MW_COLS], bf16, tag="mlpw")
            nc.gpsimd.dma_start(mlpw[:], mlpw_d[:])
            bd2h = wp.tile([RT, 1], f32, tag="bd2h")
            nc.gpsimd.dma_start(bd2h[:], bd2h_d[:])

            # ---- ACT table warm (Exp/Tanh/Relu/Identity share one set) ----
            warm = sb.tile([1, 1], f32, tag="warm")
            nc.vector.memset(warm[:], 0.0)
            warm_o = sb.tile([1, 1], f32, tag="warmo")
            nc.scalar.activation(warm_o[:], warm[:], AF.Exp)

            # ---- ones rows for augmented operands ----
            tsum = sb.tile([65, RT], bf16, tag="tsum")
            nc.vector.memset(tsum[64:65, :], 1.0)
            r0a = sb.tile([65, SHARD], bf16, tag="r0a")
            nc.vector.memset(r0a[64:65, :], 1.0)
            v0a = sb.tile([65, SHARD], bf16, tag="v0a")
            nc.vector.memset(v0a[64:65, :], 1.0)

            def leaky(out_ap, in_ap):
                nc.vector.scalar_tensor_tensor(out=out_ap, in0=in_ap, scalar=SLOPE,
                                               in1=in_ap, op0=ALU.mult, op1=ALU.max)

            # ---- prologue: h1 chain + folded C path ----
            with nc.allow_low_precision(reason="3-way sum of bf16 agent vectors"):
                nc.vector.reduce_sum(
                    tsum[0:64, :],
                    pro2[0:64, PRO_TA:PRO_TA + RT * APT].rearrange("p (t a) -> p t a", a=APT),
                    axis=AX.X)
            h1_ps = ps.tile([H, 1], f32, tag="gl", bufs=2, name="h1_ps")
            nc.tensor.matmul(h1_ps[:], pro1[0:65, PRO_WSELF:PRO_WSELF + H],
                             pro1[0:65, PRO_HID:PRO_HID + 1], start=True, stop=True)
            h1 = sb.tile([H, 1], bf16, tag="h1", bufs=2)
            nc.vector.tensor_copy(h1[:], h1_ps[:])

            C_ps = ps.tile([H, RT], f32, tag="grq", padded_shape=PAD_BIG, name="C_ps")
            for t in range(RT):
                nc.tensor.matmul(C_ps[:, t:t + 1],
                                 pro2[0:65, PRO_WC + H * t:PRO_WC + H * (t + 1)],
                                 tsum[:, t:t + 1], start=True, stop=True)
            C_sb = sb.tile([H, RT], f32, tag="Csb")
            nc.vector.tensor_copy(C_sb[:], C_ps[:])

            # serial merge chain -> h1 (node 0 embedding)
            chain_ps = ps.tile([H, 1], f32, tag="grq", padded_shape=PAD_BIG,
                               name="chain_ps")
            for t in range(RT):
                nc.tensor.matmul(chain_ps[:], pro1[0:64, PRO_WML:PRO_WML + H], h1[:],
                                 start=True, stop=True)
                u = sb.tile([H, 1], bf16, tag="u", bufs=2)
                nc.vector.tensor_scalar_add(u[:], chain_ps[:], C_sb[:, t:t + 1])
                h1n = sb.tile([H, 1], bf16, tag="h1", bufs=2)
                leaky(h1n[:], u[:])
                h1 = h1n

            # ---- g_l / g_r projections for all 1024 nodes ----
            gl_b = [ps.tile([128, N], f32, tag="gl", bufs=2, name=f"gl{b}")
                    for b in range(2)]
            gr_b0 = ps.tile([128, N], f32, tag="grE", name="gr_b0")
            gr_b1 = ps.tile([128, N], f32, tag="grq", padded_shape=PAD_BIG,
                            name="gr_b1")
            gr_b = [gr_b0, gr_b1]
            gr0c = sb.tile([128, 2], f32, tag="gr0c")
            # block 0 g_l chunks + node-0 columns + query bias first
            for lo, hi in CH:
                nc.tensor.matmul(gl_b[0][:, lo:hi], gatw[0:64, GW_WL:GW_WL + 128],
                                 gatw[0:64, lo:hi], start=True, stop=True)
            nc.tensor.matmul(gl_b[0][:, N_AMB:N], gatw[0:64, GW_WL:GW_WL + 128],
                             h1[:], start=True, stop=True)
            nc.tensor.matmul(gr_b0[:, N_AMB:N], gatw[0:64, GW_WR:GW_WR + 128],
                             h1[:], start=True, stop=True)
            nc.scalar.activation(gr0c[:, 0:1], gr_b0[:, N_AMB:N], AF.Identity)
            # block 1 g_l + node-0 columns
            for lo, hi in CH:
                nc.tensor.matmul(gl_b[1][:, lo:hi],
                                 gatw[0:64, GW_WL + 128:GW_WL + 256],
                                 gatw[0:64, lo:hi], start=True, stop=True)
            nc.tensor.matmul(gl_b[1][:, N_AMB:N], gatw[0:64, GW_WL + 128:GW_WL + 256],
                             h1[:], start=True, stop=True)
            nc.tensor.matmul(gr_b1[:, N_AMB:N], gatw[0:64, GW_WR + 128:GW_WR + 256],
                             h1[:], start=True, stop=True)
            nc.scalar.activation(gr0c[:, 1:2], gr_b1[:, N_AMB:N], AF.Identity)

            # ---- GAT pipeline, blocks overlapped across ACT/DVE/PE ----
            ssum = sb.tile([128, 2], f32, tag="ssum")
            att_u = sb.tile([128, 2], f32, tag="attu")
            pexp = [None, None]
            for b in range(2):
                tu = sb.tile([128, N], bf16, tag="tu", bufs=2, name=f"tu{b}")
                nc.scalar.activation(tu[:], gl_b[b][:], AF.Identity,
                                     bias=gr0c[:, b:b + 1])
                tsb = sb.tile([128, N], bf16, tag="tl", bufs=2, name=f"ts{b}")
                e_b = ps.tile([128, N], f32, tag="gl", bufs=2, name=f"e_b{b}")
                for lo, hi in ((0, 512), (512, N)):
                    leaky(tsb[:, lo:hi], tu[:, lo:hi])
                    nc.tensor.matmul(e_b[:, lo:hi], wexp[:], tsb[:, lo:hi],
                                     start=True, stop=True)
                # this block's g_r value chunks (needed by the weighted sum)
                for lo, hi in CH:
                    nc.tensor.matmul(gr_b[b][:, lo:hi],
                                     gatw[0:64, GW_WR + 128 * b:GW_WR + 128 * (b + 1)],
                                     gatw[0:64, lo:hi], start=True, stop=True)
                pexp[b] = sb.tile([128, N], bf16, tag="pex", bufs=2, name=f"pexp{b}")
                nc.scalar.activation(pexp[b][:], e_b[:], AF.Exp, bias=0.0,
                                     accum_out=ssum[:, b:b + 1])

            # weighted value sums (fused mul + row-accumulate)
            scr = sb.tile([128, N], bf16, tag="scr")
            for b in range(2):
                nc.vector.scalar_tensor_tensor(
                    out=scr[:], in0=pexp[b][:], scalar=1.0, in1=gr_b[b][:],
                    op0=ALU.mult, op1=ALU.mult, accum_out=att_u[:, b:b + 1])

            # softmax normalize, both blocks at once
            rs = sb.tile([128, 2], f32, tag="rs")
            nc.vector.reciprocal(rs[:], ssum[:])
            att_n = sb.tile([128, 2], bf16, tag="attn")
            nc.vector.tensor_tensor(att_n[:], att_u[:], rs[:], op=ALU.mult)

            # ---- final MLP on this core's 128-node shard ----
            # h2-independent first-layer matmul (grE slot frees after wstt0);
            # c0 = Wd0b @ h2 accumulates into the same PSUM group via a
            # broadcast-column matmul of att_n's block-sum against fused G
            att_s = sb.tile([128, 1], bf16, tag="atts")
            nc.vector.tensor_tensor(att_s[:], att_n[:, 0:1], att_n[:, 1:2],
                                    op=ALU.add)
            y0_ps = ps.tile([H, SHARD], f32, tag="grE", name="y0_ps",
                            padded_shape=PAD_BIG)
            nc.tensor.matmul(y0_ps[:], mlpw[0:65, MW_WD0A:MW_WD0A + H], mlpin[:],
                             start=True, stop=False)
            nc.tensor.matmul(y0_ps[:], mlpw[0:128, MW_G:MW_G + H],
                             att_s[:, 0:1].broadcast_to([128, SHARD]),
                             start=False, stop=True)
            # leaky(v0) = 0.8*relu(v0) + 0.2*v0 split across ACT and DVE
            nc.scalar.activation(r0a[0:64, :], y0_ps[:], AF.Relu)
            nc.vector.tensor_copy(v0a[0:64, :], y0_ps[:])
            y1_ps = ps.tile([128, SHARD], f32, tag="grq", padded_shape=PAD_BIG,
                            name="y1_ps")
            nc.tensor.matmul(y1_ps[:], mlpw[0:65, MW_WD1A:MW_WD1A + 128], r0a[:],
                             start=True, stop=False)
            nc.tensor.matmul(y1_ps[:], mlpw[0:65, MW_WD1B:MW_WD1B + 128], v0a[:],
                             start=False, stop=True)
            r1 = sb.tile([128, SHARD], bf16, tag="r1")
            nc.scalar.activation(r1[:], y1_ps[:], AF.Relu)
            v1 = sb.tile([128, SHARD], bf16, tag="v1")
            nc.vector.tensor_copy(v1[:], y1_ps[:])
            o_ps = ps.tile([RT, SHARD], f32, tag="grq", padded_shape=PAD_BIG,
                           name="o_ps")
            nc.tensor.matmul(o_ps[:], mlpw[0:128, MW_WD2A:MW_WD2A + RT], r1[:],
                             start=True, stop=False)
            nc.tensor.matmul(o_ps[:], mlpw[0:128, MW_WD2B:MW_WD2B + RT], v1[:],
                             start=False, stop=True)
            # sigmoid(z) = 0.5 + 0.5*tanh(0.5*z); the affine is applied on host
            th = sb.tile([RT, SHARD], f32, tag="th")
            nc.scalar.activation(th[:], o_ps[:], AF.Tanh, bias=bd2h[:], scale=0.5)
            nc.sync.dma_start(outT_d[:], th[:])

    nc.compile()
    return nc


def _prep_inputs(inputs):
    import ml_dtypes
    f32 = np.float32
    bf16 = ml_dtypes.bfloat16

    def bf(a):
        return np.ascontiguousarray(np.asarray(a, f32), dtype=f32).astype(bf16)

    hidden = np.asarray(inputs["hidden"], f32)
    ambiguous = np.asarray(inputs["ambiguous"], f32)
    type_agents = np.asarray(inputs["type_agents"], f32)
    W_self = np.asarray(inputs["W_self"], f32)
    b_self = np.asarray(inputs["b_self"], f32)
    W_merge = np.asarray(inputs["W_merge"], f32)
    b_merge = np.asarray(inputs["b_merge"], f32)
    W_trans = np.asarray(inputs["W_trans"], f32)
    b_trans = np.asarray(inputs["b_trans"], f32)
    W_l = np.asarray(inputs["W_l"], f32)
    W_r = np.asarray(inputs["W_r"], f32)
    w_attn = np.asarray(inputs["w_attn"], f32)
    Wd0 = np.asarray(inputs["Wd0"], f32)
    bd0 = np.asarray(inputs["bd0"], f32)
    Wd1 = np.asarray(inputs["Wd1"], f32)
    bd1 = np.asarray(inputs["bd1"], f32)
    Wd2 = np.asarray(inputs["Wd2"], f32)
    bd2 = np.asarray(inputs["bd2"], f32)

    WmR = W_merge[:, H:]

    # pro packs
    pro1 = np.zeros((65, PRO1_COLS), f32)
    pro1[0:64, PRO_WSELF:PRO_WSELF + H] = W_self.T
    pro1[64, PRO_WSELF:PRO_WSELF + H] = b_self
    pro1[0:64, PRO_WML:PRO_WML + H] = W_merge[:, :H].T
    pro1[0:64, PRO_HID] = hidden[0]
    pro1[64, PRO_HID] = 1.0
    pro2 = np.zeros((65, PRO2_COLS), f32)
    for t in range(RT):
        # C_t = WmR @ (W_trans[t]/3 @ tsum_t + b_trans[t]) + b_merge
        WC = WmR @ W_trans[t] / APT
        bC = WmR @ b_trans[t] + b_merge
        pro2[0:64, PRO_WC + H * t:PRO_WC + H * (t + 1)] = WC.T
        pro2[64, PRO_WC + H * t:PRO_WC + H * (t + 1)] = bC
    pro2[0:64, PRO_TA:PRO_TA + RT * APT] = type_agents.reshape(RT * APT, H).T

    # gatw pack [64, 1536]; column 1023 is the h1 slot (filled on device)
    gatw = np.zeros((64, GW_COLS), f32)
    gatw[:, GW_AMB:GW_AMB + N_AMB] = ambiguous.T
    gatw[:, GW_WL:GW_WL + 256] = W_l.T
    gatw[:, GW_WR:GW_WR + 256] = W_r.T

    # block-diagonal w_attn (per-head logit replicated across 64 rows)
    wexp = np.zeros((128, 128), f32)
    for hh in range(2):
        wexp[hh * 64:(hh + 1) * 64, hh * 64:(hh + 1) * 64] = w_attn[:, None]

    # mlpw pack [128, 392]
    mlpw = np.zeros((128, MW_COLS), f32)
    G = 0.25 * Wd0[:, H:].T  # fold (mean over 4 heads) fused into Wd0b
    mlpw[0:64, MW_G:MW_G + H] = G
    mlpw[64:128, MW_G:MW_G + H] = G
    mlpw[0:64, MW_WD1A:MW_WD1A + 128] = (1 - SLOPE) * Wd1.T
    mlpw[64, MW_WD1A:MW_WD1A + 128] = bd1
    mlpw[0:64, MW_WD1B:MW_WD1B + 128] = SLOPE * Wd1.T
    mlpw[0:64, MW_WD0A:MW_WD0A + H] = Wd0[:, :H].T
    mlpw[64, MW_WD0A:MW_WD0A + H] = bd0
    mlpw[0:128, MW_WD2A:MW_WD2A + RT] = (1 - SLOPE) * Wd2.T
    mlpw[0:128, MW_WD2B:MW_WD2B + RT] = SLOPE * Wd2.T

    shared = {
        "pro1": bf(pro1),
        "pro2": bf(pro2),
        "gatw": bf(gatw),
        "wexp": bf(wexp),
        "mlpw": bf(mlpw),
        "bd2h": np.ascontiguousarray(0.5 * bd2.reshape(RT, 1), f32),
    }
    amb_pad = np.zeros((65, NCORES * SHARD), f32)
    amb_pad[0:64, :N_AMB] = ambiguous.T
    amb_pad[64, :] = 1.0
    in_maps = []
    for cidx in range(NCORES):
        m = dict(shared)
        m["mlpin"] = bf(amb_pad[:, cidx * SHARD:(cidx + 1) * SHARD])
        in_maps.append(m)
    return in_maps


def kernel(**inputs) -> np.ndarray:
    global _compiled
    if _compiled is None:
        _compiled = _build()
    nc = _compiled
    from concourse import bass_utils

    in_maps = _prep_inputs(inputs)
    res = bass_utils.run_bass_kernel_spmd(nc, in_maps, core_ids=list(range(NCORES)))
    out = np.empty((N_AMB, RT), np.float32)
    for cidx in range(NCORES):
        lo = cidx * SHARD
        hi = min(lo + SHARD, N_AMB)
        # device returns tanh(z/2); sigmoid(z) = 0.5 + 0.5*tanh(z/2)
        out[lo:hi, :] = 0.5 + 0.5 * res.results[cidx]["outT"][:, :hi - lo].T
    return out


# revision 19
# speedup vs baseline: 1.0139x; 1.0139x over previous
"""Trainium2 Bass kernel for nn_MlroleNode_64716567216639 (GAT message passing).

Math note: the reference computes a dense NxN GATv2 attention but only row 0
of the output (gat_out[0]) feeds the final MLP, so this kernel computes just
that row: e[j,h] = leaky(g_l[j] + g_r[0]) . w_attn over the 1024 source nodes,
softmax, weighted sum of g_r, then the 3-layer type-define MLP over the 1023
ambiguous nodes.

Structure: node 0's embedding h1 (serial role-routing prologue) is written
into column 1023 of the on-chip node matrix, so the per-source-node work is a
single [128,1024] pipeline; only three 1-column matmuls depend on the
prologue. All matmuls and large elementwise ops run in bf16 (fp32 PSUM
accumulation); biases are folded into augmented weight rows and the per-role
merge bias C = WmR@(Wt/3 @ sum(agents) + bt) + bm is collapsed into a single
matmul host-side. MLP leaky-relus use leaky(v) = 0.8*relu(v) + 0.2*v with the
0.8/0.2 folded into split weight copies so ACT (relu) and DVE (copy) run in
parallel. The sigmoid is computed as tanh on-device with the affine
0.5+0.5*x applied on host. The GAT row-0 computation is replicated on all 8
cores; the final MLP is sharded 128 nodes per core.
"""
import numpy as np

H = 64
N_AMB = 1023
N = 1024            # 1023 ambiguous + node 0 (in column 1023)
HEADS = 4
HID = 64
RT = 4
APT = 3
SLOPE = 0.2
NCORES = 8
SHARD = 128

# "pro1" [65, 129]: chain weights; "pro2" [65, 268]: folded C-path weights
PRO_WSELF = 0          # WselfT_aug [65,64]
PRO_WML = 64           # WmLT [64,64]
PRO_HID = 128          # hidc_aug [65,1]
PRO1_COLS = 129
PRO_WC = 0             # 4x WC_t_aug [65,64]: C_t = (WmR@Wt_t/3) @ tsum + bC_t
PRO_TA = 256           # ta [64,12]
PRO2_COLS = 268

# column offsets in the packed "gatw" tensor [64, 1536]
GW_AMB = 0             # ambT [64,1023] + h1 slot col 1023
GW_WL = 1024           # WlT [64,256] (2 blocks of 128)
GW_WR = 1280           # WrT [64,256]
GW_COLS = 1536

# column offsets in the packed "mlpw" tensor [128, 392]
MW_G = 0               # G = 0.25*[Wd0b.T; Wd0b.T] [128,64]
MW_WD1A = 64           # [0.8*Wd1.T; bd1] [65,128]
MW_WD1B = 192          # [0.2*Wd1.T; 0]  [65,128]
MW_WD0A = 320          # Wd0a_aug [65,64]
MW_WD2A = 384          # 0.8*Wd2.T [128,4]
MW_WD2B = 388          # 0.2*Wd2.T [128,4]
MW_COLS = 392

_compiled = None


def _build():
    import concourse.tile as tile
    from concourse import bacc, mybir

    f32 = mybir.dt.float32
    bf16 = mybir.dt.bfloat16
    AF = mybir.ActivationFunctionType
    ALU = mybir.AluOpType
    AX = mybir.AxisListType

    nc = bacc.Bacc("TRN2", target_bir_lowering=False, debug=False,
                   enable_asserts=False, num_devices=NCORES)

    pro1_d = nc.dram_tensor("pro1", [65, PRO1_COLS], bf16, kind="ExternalInput").ap()
    pro2_d = nc.dram_tensor("pro2", [65, PRO2_COLS], bf16, kind="ExternalInput").ap()
    gatw_d = nc.dram_tensor("gatw", [64, GW_COLS], bf16, kind="ExternalInput").ap()
    wexp_d = nc.dram_tensor("wexp", [128, 128], bf16, kind="ExternalInput").ap()
    mlpw_d = nc.dram_tensor("mlpw", [128, MW_COLS], bf16, kind="ExternalInput").ap()
    mlpin_d = nc.dram_tensor("mlpin", [65, SHARD], bf16, kind="ExternalInput").ap()
    bd2h_d = nc.dram_tensor("bd2h", [RT, 1], f32, kind="ExternalInput").ap()
    outT_d = nc.dram_tensor("outT", [RT, SHARD], f32, kind="ExternalOutput").ap()

    # column chunks: [0:512), [512:1023) independent of h1; col 1023 is h1
    CH = ((0, 512), (512, N_AMB))
    PAD_BIG = [128, 1024]   # 2 PSUM banks

    with tile.TileContext(nc) as tc:
        with tc.tile_pool(name="wp", bufs=1) as wp, \
             tc.tile_pool(name="sb", bufs=1) as sb, \
             tc.tile_pool(name="ps", bufs=1, space="PSUM") as ps:

            # ---- input DMAs, critical-first, split across two queues ----
            pro2 = wp.tile([65, PRO2_COLS], bf16, tag="pro2")
            nc.sync.dma_start(pro2[:], pro2_d[:])
            pro1 = wp.tile([65, PRO1_COLS], bf16, tag="pro1")
            nc.sync.dma_start(pro1[:], pro1_d[:])
            gatw = wp.tile([64, GW_COLS], bf16, tag="gatw")
            nc.sync.dma_start(gatw[:], gatw_d[:])
            wexp = wp.tile([128, 128], bf16, tag="wexp")
            nc.sync.dma_start(wexp[:], wexp_d[:])
            mlpin = wp.tile([65, SHARD], bf16, tag="mlpin")
            nc.gpsimd.dma_start(mlpin[:], mlpin_d[:])
            mlpw = wp.tile([128, MW_COLS], bf16, tag="mlpw")
            nc.gpsimd.dma_start(mlpw[:], mlpw_d[:])
            bd2h = wp.tile([RT, 1], f32, tag="bd2h")
            nc.gpsimd.dma_start(bd2h[:], bd2h_d[:])

            # ---- ACT table warm (Exp/Tanh/Relu/Identity share one set) ----
            warm = sb.tile([1, 1], f32, tag="warm")
            nc.vector.memset(warm[:], 0.0)
            warm_o = sb.tile([1, 1], f32, tag="warmo")
            nc.scalar.activation(warm_o[:], warm[:], AF.Exp)

            # ---- ones rows for augmented operands ----
            tsum = sb.tile([65, RT], bf16, tag="tsum")
            nc.vector.memset(tsum[64:65, :], 1.0)
            r0a = sb.tile([65, SHARD], bf16, tag="r0a")
            nc.vector.memset(r0a[64:65, :], 1.0)
            v0a = sb.tile([65, SHARD], bf16, tag="v0a")
            nc.vector.memset(v0a[64:65, :], 1.0)

            def leaky(out_ap, in_ap):
                nc.vector.scalar_tensor_tensor(out=out_ap, in0=in_ap, scalar=SLOPE,
                                               in1=in_ap, op0=ALU.mult, op1=ALU.max)

            # ---- prologue: h1 chain + folded C path ----
            with nc.allow_low_precision(reason="3-way sum of bf16 agent vectors"):
                nc.vector.reduce_sum(
                    tsum[0:64, :],
                    pro2[0:64, PRO_TA:PRO_TA + RT * APT].rearrange("p (t a) -> p t a", a=APT),
                    axis=AX.X)
            h1_ps = ps.tile([H, 1], f32, tag="gl", bufs=2, name="h1_ps")
            nc.tensor.matmul(h1_ps[:], pro1[0:65, PRO_WSELF:PRO_WSELF + H],
                             pro1[0:65, PRO_HID:PRO_HID + 1], start=True, stop=True)
            h1 = sb.tile([H, 1], bf16, tag="h1", bufs=2)
            nc.vector.tensor_copy(h1[:], h1_ps[:])

            C_ps = ps.tile([H, RT], f32, tag="grq", padded_shape=PAD_BIG, name="C_ps")
            for t in range(RT):
                nc.tensor.matmul(C_ps[:, t:t + 1],
                                 pro2[0:65, PRO_WC + H * t:PRO_WC + H * (t + 1)],
                                 tsum[:, t:t + 1], start=True, stop=True)
            C_sb = sb.tile([H, RT], f32, tag="Csb")
            nc.vector.tensor_copy(C_sb[:], C_ps[:])

            # serial merge chain -> h1 (node 0 embedding)
            chain_ps = ps.tile([H, 1], f32, tag="grq", padded_shape=PAD_BIG,
                               name="chain_ps")
            for t in range(RT):
                nc.tensor.matmul(chain_ps[:], pro1[0:64, PRO_WML:PRO_WML + H], h1[:],
                                 start=True, stop=True)
                u = sb.tile([H, 1], bf16, tag="u", bufs=2)
                nc.vector.tensor_scalar_add(u[:], chain_ps[:], C_sb[:, t:t + 1])
                h1n = sb.tile([H, 1], bf16, tag="h1", bufs=2)
                leaky(h1n[:], u[:])
                h1 = h1n

            # ---- g_l / g_r projections for all 1024 nodes ----
            gl_b = [ps.tile([128, N], f32, tag="gl", bufs=2, name=f"gl{b}")
                    for b in range(2)]
            gr_b0 = ps.tile([128, N], f32, tag="grE", name="gr_b0")
            gr_b1 = ps.tile([128, N], f32, tag="grq", padded_shape=PAD_BIG,
                            name="gr_b1")
            gr_b = [gr_b0, gr_b1]
            gr0c = sb.tile([128, 2], f32, tag="gr0c")
            # block 0 g_l chunks, then all node-0 columns + query biases
            for lo, hi in CH:
                nc.tensor.matmul(gl_b[0][:, lo:hi], gatw[0:64, GW_WL:GW_WL + 128],
                                 gatw[0:64, lo:hi], start=True, stop=True)
            nc.tensor.matmul(gl_b[0][:, N_AMB:N], gatw[0:64, GW_WL:GW_WL + 128],
                             h1[:], start=True, stop=True)
            nc.tensor.matmul(gr_b0[:, N_AMB:N], gatw[0:64, GW_WR:GW_WR + 128],
                             h1[:], start=True, stop=True)
            nc.tensor.matmul(gl_b[1][:, N_AMB:N], gatw[0:64, GW_WL + 128:GW_WL + 256],
                             h1[:], start=True, stop=True)
            nc.tensor.matmul(gr_b1[:, N_AMB:N], gatw[0:64, GW_WR + 128:GW_WR + 256],
                             h1[:], start=True, stop=True)
            nc.vector.tensor_copy(gr0c[:, 0:1], gr_b0[:, N_AMB:N])
            nc.vector.tensor_copy(gr0c[:, 1:2], gr_b1[:, N_AMB:N])
            # block 1 g_l chunks
            for lo, hi in CH:
                nc.tensor.matmul(gl_b[1][:, lo:hi],
                                 gatw[0:64, GW_WL + 128:GW_WL + 256],
                                 gatw[0:64, lo:hi], start=True, stop=True)

            # ---- GAT pipeline, blocks overlapped across ACT/DVE/PE ----
            ssum = sb.tile([128, 2], f32, tag="ssum")
            att_u = sb.tile([128, 2], f32, tag="attu")
            pexp = [None, None]
            for b in range(2):
                tu = sb.tile([128, N], bf16, tag="tu", bufs=2, name=f"tu{b}")
                nc.scalar.activation(tu[:], gl_b[b][:], AF.Identity,
                                     bias=gr0c[:, b:b + 1])
                tsb = sb.tile([128, N], bf16, tag="tl", bufs=2, name=f"ts{b}")
                e_b = ps.tile([128, N], f32, tag="gl", bufs=2, name=f"e_b{b}")
                for lo, hi in ((0, 512), (512, N)):
                    leaky(tsb[:, lo:hi], tu[:, lo:hi])
                    nc.tensor.matmul(e_b[:, lo:hi], wexp[:], tsb[:, lo:hi],
                                     start=True, stop=True)
                # this block's g_r value chunks (needed by the weighted sum)
                for lo, hi in CH:
                    nc.tensor.matmul(gr_b[b][:, lo:hi],
                                     gatw[0:64, GW_WR + 128 * b:GW_WR + 128 * (b + 1)],
                                     gatw[0:64, lo:hi], start=True, stop=True)
                pexp[b] = sb.tile([128, N], bf16, tag="pex", bufs=2, name=f"pexp{b}")
                nc.scalar.activation(pexp[b][:], e_b[:], AF.Exp, bias=0.0,
                                     accum_out=ssum[:, b:b + 1])

            # weighted value sums (fused mul + row-accumulate)
            scr = sb.tile([128, N], bf16, tag="scr")
            for b in range(2):
                nc.vector.scalar_tensor_tensor(
                    out=scr[:], in0=pexp[b][:], scalar=1.0, in1=gr_b[b][:],
                    op0=ALU.mult, op1=ALU.mult, accum_out=att_u[:, b:b + 1])

            # softmax normalize, both blocks at once
            rs = sb.tile([128, 2], f32, tag="rs")
            nc.vector.reciprocal(rs[:], ssum[:])
            att_n = sb.tile([128, 2], bf16, tag="attn")
            nc.vector.tensor_tensor(att_n[:], att_u[:], rs[:], op=ALU.mult)

            # ---- final MLP on this core's 128-node shard ----
            # h2-independent first-layer matmul (grE slot frees after wstt0);
            # c0 = Wd0b @ h2 accumulates into the same PSUM group via a
            # broadcast-column matmul of att_n's block-sum against fused G
            att_s = sb.tile([128, 1], bf16, tag="atts")
            nc.vector.tensor_tensor(att_s[:], att_n[:, 0:1], att_n[:, 1:2],
                                    op=ALU.add)
            y0_ps = ps.tile([H, SHARD], f32, tag="grE", name="y0_ps",
                            padded_shape=PAD_BIG)
            nc.tensor.matmul(y0_ps[:], mlpw[0:65, MW_WD0A:MW_WD0A + H], mlpin[:],
                             start=True, stop=False)
            nc.tensor.matmul(y0_ps[:], mlpw[0:128, MW_G:MW_G + H],
                             att_s[:, 0:1].broadcast_to([128, SHARD]),
                             start=False, stop=True)
            # leaky(v0) = 0.8*relu(v0) + 0.2*v0 split across ACT and DVE
            nc.scalar.activation(r0a[0:64, :], y0_ps[:], AF.Relu)
            nc.vector.tensor_copy(v0a[0:64, :], y0_ps[:])
            y1_ps = ps.tile([128, SHARD], f32, tag="grq", padded_shape=PAD_BIG,
                            name="y1_ps")
            nc.tensor.matmul(y1_ps[:], mlpw[0:65, MW_WD1A:MW_WD1A + 128], r0a[:],
                             start=True, stop=False)
            nc.tensor.matmul(y1_ps[:], mlpw[0:65, MW_WD1B:MW_WD1B + 128], v0a[:],
                             start=False, stop=True)
            r1 = sb.tile([128, SHARD], bf16, tag="r1")
            nc.scalar.activation(r1[:], y1_ps[:], AF.Relu)
            v1 = sb.tile([128, SHARD], bf16, tag="v1")
            nc.vector.tensor_copy(v1[:], y1_ps[:])
            o_ps = ps.tile([RT, SHARD], f32, tag="grq", padded_shape=PAD_BIG,
                           name="o_ps")
            nc.tensor.matmul(o_ps[:], mlpw[0:128, MW_WD2A:MW_WD2A + RT], r1[:],
                             start=True, stop=False)
            nc.tensor.matmul(o_ps[:], mlpw[0:128, MW_WD2B:MW_WD2B + RT], v1[:],
                             start=False, stop=True)
            # sigmoid(z) = 0.5 + 0.5*tanh(0.5*z); the affine is applied on host
            th = sb.tile([RT, SHARD], f32, tag="th")
            nc.scalar.activation(th[:], o_ps[:], AF.Tanh, bias=bd2h[:], scale=0.5)
            nc.sync.dma_start(outT_d[:], th[:])

    nc.compile()
    return nc


def _prep_inputs(inputs):
    import ml_dtypes
    f32 = np.float32
    bf16 = ml_dtypes.bfloat16

    def bf(a):
        return np.ascontiguousarray(np.asarray(a, f32), dtype=f32).astype(bf16)

    hidden = np.asarray(inputs["hidden"], f32)
    ambiguous = np.asarray(inputs["ambiguous"], f32)
    type_agents = np.asarray(inputs["type_agents"], f32)
    W_self = np.asarray(inputs["W_self"], f32)
    b_self = np.asarray(inputs["b_self"], f32)
    W_merge = np.asarray(inputs["W_merge"], f32)
    b_merge = np.asarray(inputs["b_merge"], f32)
    W_trans = np.asarray(inputs["W_trans"], f32)
    b_trans = np.asarray(inputs["b_trans"], f32)
    W_l = np.asarray(inputs["W_l"], f32)
    W_r = np.asarray(inputs["W_r"], f32)
    w_attn = np.asarray(inputs["w_attn"], f32)
    Wd0 = np.asarray(inputs["Wd0"], f32)
    bd0 = np.asarray(inputs["bd0"], f32)
    Wd1 = np.asarray(inputs["Wd1"], f32)
    bd1 = np.asarray(inputs["bd1"], f32)
    Wd2 = np.asarray(inputs["Wd2"], f32)
    bd2 = np.asarray(inputs["bd2"], f32)

    WmR = W_merge[:, H:]

    # pro packs
    pro1 = np.zeros((65, PRO1_COLS), f32)
    pro1[0:64, PRO_WSELF:PRO_WSELF + H] = W_self.T
    pro1[64, PRO_WSELF:PRO_WSELF + H] = b_self
    pro1[0:64, PRO_WML:PRO_WML + H] = W_merge[:, :H].T
    pro1[0:64, PRO_HID] = hidden[0]
    pro1[64, PRO_HID] = 1.0
    pro2 = np.zeros((65, PRO2_COLS), f32)
    for t in range(RT):
        # C_t = WmR @ (W_trans[t]/3 @ tsum_t + b_trans[t]) + b_merge
        WC = WmR @ W_trans[t] / APT
        bC = WmR @ b_trans[t] + b_merge
        pro2[0:64, PRO_WC + H * t:PRO_WC + H * (t + 1)] = WC.T
        pro2[64, PRO_WC + H * t:PRO_WC + H * (t + 1)] = bC
    pro2[0:64, PRO_TA:PRO_TA + RT * APT] = type_agents.reshape(RT * APT, H).T

    # gatw pack [64, 1536]; column 1023 is the h1 slot (filled on device)
    gatw = np.zeros((64, GW_COLS), f32)
    gatw[:, GW_AMB:GW_AMB + N_AMB] = ambiguous.T
    gatw[:, GW_WL:GW_WL + 256] = W_l.T
    gatw[:, GW_WR:GW_WR + 256] = W_r.T

    # block-diagonal w_attn (per-head logit replicated across 64 rows)
    wexp = np.zeros((128, 128), f32)
    for hh in range(2):
        wexp[hh * 64:(hh + 1) * 64, hh * 64:(hh + 1) * 64] = w_attn[:, None]

    # mlpw pack [128, 392]
    mlpw = np.zeros((128, MW_COLS), f32)
    G = 0.25 * Wd0[:, H:].T  # fold (mean over 4 heads) fused into Wd0b
    mlpw[0:64, MW_G:MW_G + H] = G
    mlpw[64:128, MW_G:MW_G + H] = G
    mlpw[0:64, MW_WD1A:MW_WD1A + 128] = (1 - SLOPE) * Wd1.T
    mlpw[64, MW_WD1A:MW_WD1A + 128] = bd1
    mlpw[0:64, MW_WD1B:MW_WD1B + 128] = SLOPE * Wd1.T
    mlpw[0:64, MW_WD0A:MW_WD0A + H] = Wd0[:, :H].T
    mlpw[64, MW_WD0A:MW_WD0A + H] = bd0
    mlpw[0:128, MW_WD2A:MW_WD2A + RT] = (1 - SLOPE) * Wd2.T
    mlpw[0:128, MW_WD2B:MW_WD2B + RT] = SLOPE * Wd2.T

    shared = {
        "pro1": bf(pro1),
        "pro2": bf(pro2),
        "gatw": bf(gatw),
        "wexp": bf(wexp),
        "mlpw": bf(mlpw),
        "bd2h": np.ascontiguousarray(0.5 * bd2.reshape(RT, 1), f32),
    }
    amb_pad = np.zeros((65, NCORES * SHARD), f32)
    amb_pad[0:64, :N_AMB] = ambiguous.T
    amb_pad[64, :] = 1.0
    in_maps = []
    for cidx in range(NCORES):
        m = dict(shared)
        m["mlpin"] = bf(amb_pad[:, cidx * SHARD:(cidx + 1) * SHARD])
        in_maps.append(m)
    return in_maps


def kernel(**inputs) -> np.ndarray:
    global _compiled
    if _compiled is None:
        _compiled = _build()
    nc = _compiled
    from concourse import bass_utils

    in_maps = _prep_inputs(inputs)
    res = bass_utils.run_bass_kernel_spmd(nc, in_maps, core_ids=list(range(NCORES)))
    out = np.empty((N_AMB, RT), np.float32)
    for cidx in range(NCORES):
        lo = cidx * SHARD
        hi = min(lo + SHARD, N_AMB)
        # device returns tanh(z/2); sigmoid(z) = 0.5 + 0.5*tanh(z/2)
        out[lo:hi, :] = 0.5 + 0.5 * res.results[cidx]["outT"][:, :hi - lo].T
    return out
